# revision 35
# baseline (speedup 1.0000x reference)
"""Fused 8-layer transformer (pre-LN, MHA + FFN) for TRN2, data-parallel
over batch across 8 NeuronCores.

Layout strategy: feature-major ("transposed") activations resident in SBUF.
x is kept as xT[feature 128-part, hc, token] fp32. All matmuls contract over
features on the partition dim, so no PE transposes are needed anywhere.
LayerNorm stats (over features = partitions) are computed with ones-matmuls
in float32r. Attention works per (batch, head) on 256x256 score tiles in
[k, q] layout; softmax denominators come from an augmented ones-column in v.
exp(attn_bias) is precomputed on host so softmax is exp(s) * eb (no max
subtraction needed: scores are tiny by construction).
"""

import numpy as np
import ml_dtypes
from contextlib import ExitStack

import bass_rust
import concourse.bass as bass
import concourse.tile as tile
from concourse import mybir
from concourse.bass_utils import run_bass_kernel_spmd

BF16 = ml_dtypes.bfloat16

B, N, H, HEADS, DH, F, L = 32, 256, 512, 8, 64, 2048, 8
NC = 8
BL = B // NC            # local batch = 4
T = BL * N              # local tokens = 1024
EPS = 1e-5

FP32 = mybir.dt.float32
BF = mybir.dt.bfloat16
F32R = mybir.dt.float32r


def _legalize_sync(nc):
    # This walrus codegen encodes at most 1 sem wait + 1 sem update per
    # instruction; the Tile scheduler emits more at cross-engine joins.
    # Hoist excess waits onto same-engine NoOps inserted just before
    # (queues are in-order, so this preserves the happens-before) and
    # excess updates onto NoOps just after.
    uid = 0
    for fn in nc.m.functions:
        for blk in fn.blocks:
            out = []
            changed = False
            for ins in blk.instructions:
                si = ins.sync_info
                if si is not None and (len(si.on_wait) > 1 or len(si.on_update) > 1):
                    waits = list(si.on_wait)
                    upds = list(si.on_update)
                    for w in waits[:-1]:
                        uid += 1
                        nop = bass_rust.InstNoOp(name=f"LGLW-{uid}", engine=ins.engine)
                        nop.sync_info = mybir.SyncInfo(on_wait=[w], on_update=[])
                        out.append(nop)
                    post = []
                    if len(upds) > 1:
                        opname = type(ins).__name__
                        assert "DMA" not in opname and "Dma" not in opname, ins.name
                        for u in upds[1:]:
                            uid += 1
                            nop = bass_rust.InstNoOp(
                                name=f"LGLU-{uid}", engine=ins.engine)
                            nop.sync_info = mybir.SyncInfo(on_wait=[], on_update=[u])
                            post.append(nop)
                        upds = upds[:1]
                    ins.sync_info = mybir.SyncInfo(on_wait=waits[-1:], on_update=upds)
                    out.append(ins)
                    out.extend(post)
                    changed = True
                else:
                    out.append(ins)
            if changed:
                blk.instructions = out


def _build_nc():
    nc = bass.Bass("TRN2", target_bir_lowering=False, debug=False)
    AF = mybir.ActivationFunctionType
    OP = mybir.AluOpType

    def din(name, shape, dt):
        return nc.dram_tensor(name, shape, dt, kind="ExternalInput").ap()

    x_in = din("x_in", [128, 4, T], FP32)
    eb_in = din("eb_in", [BL * HEADS, 128, 512], BF)
    wq_in = din("wq_in", [L, 128, 2048], BF)
    wk_in = din("wk_in", [L, 128, 2048], BF)
    wv_in = din("wv_in", [L, 128, 2048], BF)
    wo_in = din("wo_in", [L, 128, 2048], BF)
    w1_in = din("w1_in", [L, 128, 8192], BF)
    w2_in = din("w2_in", [L, 128, 8192], BF)
    bq_in = din("bq_in", [L, 128, 4], FP32)
    bk_in = din("bk_in", [L, 128, 4], FP32)
    b1_in = din("b1_in", [L, 128, 16], FP32)
    bo_in = din("bo_in", [L, 128, 4], FP32)
    b2_in = din("b2_in", [L, 128, 4], FP32)
    sel_in = din("sel_in", [32, 16, 128], BF)
    y_out = nc.dram_tensor("y_out", [128, 4, T], FP32, kind="ExternalOutput").ap()

    with ExitStack() as stk:
        tc = stk.enter_context(tile.TileContext(nc))
        const = stk.enter_context(tc.tile_pool(name="const", bufs=1))
        wts = stk.enter_context(tc.tile_pool(name="wts", bufs=1))
        work = stk.enter_context(tc.tile_pool(name="work", bufs=2))
        smalls = stk.enter_context(tc.tile_pool(name="smalls", bufs=2))
        ebp = stk.enter_context(tc.tile_pool(name="ebp", bufs=3))
        pmm = stk.enter_context(tc.tile_pool(name="pmm", bufs=6, space="PSUM"))
        pacc = stk.enter_context(tc.tile_pool(name="pacc", bufs=2, space="PSUM"))

        xT = const.tile([128, 4, T], FP32, tag="xT")
        ones = const.tile([128, 1], BF, tag="ones")
        nc.vector.memset(ones, 1.0 / H)
        eps_t = const.tile([1, 1], FP32, tag="eps")
        nc.vector.memset(eps_t, EPS)
        ones_r = const.tile([1, 128], BF, tag="ones_r")
        nc.vector.memset(ones_r, 1.0)
        # per-(batch, head-pair) selector for the deferred attention
        # normalization: picks den rows (8b+2hc, 8b+2hc+1) into the two
        # 64-partition halves (host-built; engines can't memset at
        # arbitrary base partitions)
        sel_all = const.tile([32, 16, 128], BF, tag="sel_all")
        nc.sync.dma_start(sel_all, sel_in)
        # v_aug[kt within chunk, token-chunk, head, d | ones-column]
        v_aug = const.tile([128, 8, HEADS, 65], BF, tag="v_aug")
        nc.vector.memset(v_aug[:, :, :, 64:65], 1.0)

        nc.sync.dma_start(xT, x_in)

        def layernorm(tag):
            yT = work.tile([128, 4, T], BF, tag="yT", name=tag, bufs=2)
            for th in range(2):
                tsl = slice(th * 512, (th + 1) * 512)
                ps_s = pmm.tile([128, 512], FP32, tag="mm", name="ps_s")
                ps_q = pmm.tile([128, 512], FP32, tag="mm", name="ps_q")
                for hc in range(4):
                    xb = work.tile([128, 512], BF, tag="xb", name="xb")
                    nc.vector.tensor_scalar_add(xb, xT[:, hc, tsl], 0.0)
                    sq = work.tile([128, 512], BF, tag="sq", name="sq")
                    nc.vector.tensor_mul(sq, xT[:, hc, tsl], xT[:, hc, tsl])
                    nc.tensor.matmul(ps_s[0:1, :], ones, xb,
                                     start=hc == 0, stop=hc == 3)
                    nc.tensor.matmul(ps_q[0:1, :], ones, sq,
                                     start=hc == 0, stop=hc == 3)
                # ps_s[0] = mean, ps_q[0] = E[x^2] (ones pre-scaled by 1/H)
                msq = smalls.tile([1, 512], FP32, tag="msq", name="msq")
                nc.scalar.activation(msq, ps_s[0:1, :], AF.Square)
                var = smalls.tile([1, 512], FP32, tag="var", name="var")
                nc.vector.tensor_sub(var, ps_q[0:1, :], msq)
                # rstd = exp(-0.5*ln(var+eps)); ln/exp share an ACT table set
                # with attention's Exp, and this keeps DVE reciprocal off the
                # critical path
                lnv = smalls.tile([1, 512], FP32, tag="lnv", name="lnv")
                nc.scalar.activation(lnv, var, AF.Ln, bias=eps_t)
                rstd = smalls.tile([1, 512], BF, tag="rstd", name="rstd")
                nc.scalar.activation(rstd, lnv, AF.Exp, scale=-0.5)
                mur = smalls.tile([1, 512], BF, tag="mur", name="mur")
                with nc.allow_low_precision(reason="bf16 bc-matmul operands"):
                    nc.vector.tensor_mul(mur, ps_s[0:1, :], rstd)
                r_bc = pmm.tile([128, 512], FP32, tag="mm", name="r_bc")
                m_bc = pmm.tile([128, 512], FP32, tag="mm", name="m_bc")
                nc.tensor.matmul(r_bc, ones_r, rstd, start=True, stop=True)
                nc.tensor.matmul(m_bc, ones_r, mur, start=True, stop=True)
                for hc in range(4):
                    tmp = work.tile([128, 512], FP32, tag="lntmp", name="lntmp")
                    nc.vector.tensor_mul(tmp, xT[:, hc, tsl], r_bc)
                    nc.vector.tensor_sub(yT[:, hc, tsl], tmp, m_bc)
            return yT

        for l in range(L):
            wq_t = wts.tile([128, 2048], BF, tag="wq", name="wq_t")
            wk_t = wts.tile([128, 2048], BF, tag="wk", name="wk_t")
            wv_t = wts.tile([128, 2048], BF, tag="wv", name="wv_t")
            wo_t = wts.tile([128, 2048], BF, tag="wo", name="wo_t")
            w1_t = wts.tile([128, 8192], BF, tag="w1", name="w1_t", bufs=2)
            w2_t = wts.tile([128, 8192], BF, tag="w2", name="w2_t", bufs=2)
            nc.sync.dma_start(wq_t, wq_in[l])
            nc.sync.dma_start(wk_t, wk_in[l])
            nc.sync.dma_start(wv_t, wv_in[l])
            nc.sync.dma_start(wo_t, wo_in[l])
            nc.sync.dma_start(w1_t, w1_in[l])
            nc.sync.dma_start(w2_t, w2_in[l])
            bq_t = smalls.tile([128, 4], FP32, tag="bq", name="bq_t")
            bk_t = smalls.tile([128, 4], FP32, tag="bk", name="bk_t")
            b1_t = smalls.tile([128, 16], FP32, tag="b1", name="b1_t")
            bo_t = smalls.tile([128, 4], FP32, tag="bo", name="bo_t")
            b2_t = smalls.tile([128, 4], FP32, tag="b2", name="b2_t")
            nc.sync.dma_start(bq_t, bq_in[l])
            nc.sync.dma_start(bk_t, bk_in[l])
            nc.sync.dma_start(b1_t, b1_in[l])
            nc.sync.dma_start(bo_t, bo_in[l])
            nc.sync.dma_start(b2_t, b2_in[l])

            # ---- LN1 ----
            y1 = layernorm("y1T")

            # ---- QKV (q/k feature-major, v token-major augmented) ----
            qT = work.tile([128, 4, T], BF, tag="qT", name="qT", bufs=1)
            kT = work.tile([128, 4, T], BF, tag="kT", name="kT", bufs=1)
            for oc in range(4):
                for th in range(2):
                    tsl = slice(th * 512, (th + 1) * 512)
                    pq = pmm.tile([128, 512], FP32, tag="mm", name="pq")
                    pk = pmm.tile([128, 512], FP32, tag="mm", name="pk")
                    for hc in range(4):
                        wsl = slice((hc * 4 + oc) * 128, (hc * 4 + oc + 1) * 128)
                        nc.tensor.matmul(pq, wq_t[:, wsl], y1[:, hc, tsl],
                                         start=hc == 0, stop=hc == 3)
                        nc.tensor.matmul(pk, wk_t[:, wsl], y1[:, hc, tsl],
                                         start=hc == 0, stop=hc == 3)
                    nc.scalar.activation(qT[:, oc, tsl], pq, AF.Identity,
                                         bias=bq_t[:, oc:oc + 1])
                    nc.scalar.activation(kT[:, oc, tsl], pk, AF.Identity,
                                         bias=bk_t[:, oc:oc + 1])
            for tcc in range(8):
                pv = pmm.tile([128, 512], FP32, tag="mm", name="pv")
                for hc in range(4):
                    nc.tensor.matmul(pv, y1[:, hc, tcc * 128:(tcc + 1) * 128],
                                     wv_t[:, hc * 512:(hc + 1) * 512],
                                     start=hc == 0, stop=hc == 3)
                nc.scalar.activation(v_aug[:, tcc, :, 0:64],
                                     pv.rearrange("p (h d) -> p h d", h=HEADS),
                                     AF.Copy)

            # ---- attention per (batch, head), scores in [k, q] layout ----
            ctxT = work.tile([128, 4, T], BF, tag="ctxT", name="ctxT", bufs=1)
            den = work.tile([32, 256], FP32, tag="den", name="den", bufs=1)
            def attn_post(st):
                # post-ctx ops, emitted one iteration late so they don't sit
                # between chain links on the in-order ACT/DVE queues
                pc, hp, hcq, qsl, bh = st
                nc.vector.tensor_scalar_add(ctxT[hp:hp + 64, hcq, qsl],
                                            pc[0:64, 0:256], 0.0)
                # engines can't write arbitrary base partitions: stage the
                # PSUM den row in SBUF, then DMA-scatter to partition bh
                dstg = smalls.tile([1, 256], FP32, tag="dstg",
                                   name="dstg", bufs=4)
                nc.scalar.activation(dstg, pc[64:65, 0:256], AF.Copy)
                nc.sync.dma_start(den[bh:bh + 1, :], dstg)

            prev = None
            for b in range(BL):
                for h in range(HEADS):
                    eb_t = ebp.tile([128, 512], BF, tag="eb", name="eb_t",
                                    bufs=4)
                    nc.sync.dma_start(eb_t, eb_in[b * HEADS + h])
                    hp = (h % 2) * 64
                    hcq = h // 2
                    qsl = slice(b * 256, (b + 1) * 256)
                    ps = pmm.tile([128, 512], FP32, tag="mm", name="ps")
                    for ktc in range(2):
                        nc.tensor.matmul(
                            ps[:, ktc * 256:(ktc + 1) * 256],
                            kT[hp:hp + 64, hcq, b * 256 + ktc * 128: b * 256 + (ktc + 1) * 128],
                            qT[hp:hp + 64, hcq, qsl],
                            start=True, stop=True)
                    probs = work.tile([128, 512], BF, tag="probs", name="probs", bufs=3)
                    nc.scalar.activation(probs, ps, AF.Exp)
                    nc.vector.tensor_mul(probs, probs, eb_t)
                    pc = pmm.tile([128, 512], FP32, tag="mm", name="pc")
                    for ktc in range(2):
                        nc.tensor.matmul(pc[0:65, 0:256],
                                         v_aug[:, b * 2 + ktc, h, :],
                                         probs[:, ktc * 256:(ktc + 1) * 256],
                                         start=ktc == 0, stop=ktc == 1)
                    if prev is not None:
                        attn_post(prev)
                    prev = (pc, hp, hcq, qsl, b * HEADS + h)
            attn_post(prev)
            rden = work.tile([32, 256], BF, tag="rden", name="rden", bufs=1)
            with nc.allow_low_precision(reason="bf16 bc-matmul operands"):
                nc.vector.reciprocal(rden, den)
            for b in range(BL):
                qsl = slice(b * 256, (b + 1) * 256)
                for hcq in range(4):
                    nbc = pmm.tile([128, 512], FP32, tag="mm", name="nbc")
                    nc.tensor.matmul(nbc[:, 0:256], sel_all[:, b * 4 + hcq, :],
                                     rden, start=True, stop=True)
                    nc.vector.tensor_mul(ctxT[:, hcq, qsl], ctxT[:, hcq, qsl],
                                         nbc[:, 0:256])

            # ---- out-proj + residual ----
            for oc in range(4):
                for th in range(2):
                    tsl = slice(th * 512, (th + 1) * 512)
                    po = pmm.tile([128, 512], FP32, tag="mm", name="po")
                    for hc in range(4):
                        wsl = slice((hc * 4 + oc) * 128, (hc * 4 + oc + 1) * 128)
                        nc.tensor.matmul(po, wo_t[:, wsl], ctxT[:, hc, tsl],
                                         start=hc == 0, stop=hc == 3)
                    nc.vector.scalar_tensor_tensor(
                        xT[:, oc, tsl], po, bo_t[:, oc:oc + 1], xT[:, oc, tsl],
                        OP.add, OP.add)

            # ---- LN2 + FFN ----
            y2 = layernorm("y2T")
            for th in range(2):
                tsl = slice(th * 512, (th + 1) * 512)
                # store gelu outputs so FFN2 runs oc-major with a single
                # rotating PSUM accumulator (frees banks for attention)
                g_all = work.tile([128, 16, 512], BF, tag="g_all",
                                  name="g_all", bufs=1)
                for fc in range(16):
                    pf = pmm.tile([128, 512], FP32, tag="mm", name="pf")
                    for hc in range(4):
                        wsl = slice((hc * 16 + fc) * 128, (hc * 16 + fc + 1) * 128)
                        nc.tensor.matmul(pf, w1_t[:, wsl], y2[:, hc, tsl],
                                         start=hc == 0, stop=hc == 3)
                    nc.scalar.activation(g_all[:, fc, :], pf, AF.Gelu,
                                         bias=b1_t[:, fc:fc + 1])
                for oc in range(4):
                    acc = pacc.tile([128, 512], FP32, tag="acc", name="acc",
                                    bufs=2)
                    for fc in range(16):
                        wsl = slice((fc * 4 + oc) * 128, (fc * 4 + oc + 1) * 128)
                        nc.tensor.matmul(acc, w2_t[:, wsl], g_all[:, fc, :],
                                         start=fc == 0, stop=fc == 15)
                    nc.vector.scalar_tensor_tensor(
                        xT[:, oc, tsl], acc, b2_t[:, oc:oc + 1], xT[:, oc, tsl],
                        OP.add, OP.add)

        nc.sync.dma_start(y_out, xT)
    _legalize_sync(nc)
    return nc


_NC_CACHE = {}


def _get_nc():
    if "nc" not in _NC_CACHE:
        _NC_CACHE["nc"] = _build_nc()
    return _NC_CACHE["nc"]


def _prep_inputs(x, attn_bias, ln1_s, ln1_b, wq, bq, wk, bk, wv, bv, wo, bo,
                 ln2_s, ln2_b, w1, b1, w2, b2):
    f32 = np.float32
    asf = lambda a: np.asarray(a, dtype=f32)
    x, attn_bias = asf(x), asf(attn_bias)
    ln1_s, ln1_b, ln2_s, ln2_b = asf(ln1_s), asf(ln1_b), asf(ln2_s), asf(ln2_b)
    wq, wk, wv, wo, w1, w2 = asf(wq), asf(wk), asf(wv), asf(wo), asf(w1), asf(w2)
    bq, bk, bv, bo, b1, b2 = asf(bq), asf(bk), asf(bv), asf(bo), asf(b1), asf(b2)

    scale = f32(DH ** -0.5)
    # fold LN affine into the following matmuls; fold q-scale into wq/bq;
    # fold bv through wo into bo.
    wq_f = ln1_s[:, :, None] * wq * scale
    bq_f = (bq + np.einsum("lh,lho->lo", ln1_b, wq)) * scale
    wk_f = ln1_s[:, :, None] * wk
    bk_f = bk + np.einsum("lh,lho->lo", ln1_b, wk)
    wv_f = ln1_s[:, :, None] * wv
    bv_f = bv + np.einsum("lh,lho->lo", ln1_b, wv)
    bo_f = bo + np.einsum("lh,lho->lo", bv_f, wo)
    w1_f = ln2_s[:, :, None] * w1
    b1_f = b1 + np.einsum("lh,lhf->lf", ln2_b, w1)

    def lhsT_layout(w, ncon, nout):  # w [L, ncon*128, nout*128]
        a = w.reshape(L, ncon, 128, nout, 128).transpose(0, 2, 1, 3, 4)
        return np.ascontiguousarray(a.reshape(L, 128, ncon * nout * 128)).astype(BF16)

    def rhs_layout(w):  # w [L, 512, 512] used as moving operand
        a = w.reshape(L, 4, 128, 512).transpose(0, 2, 1, 3)
        return np.ascontiguousarray(a.reshape(L, 128, 2048)).astype(BF16)

    def b_layout(bvec, nch):  # [L, nch*128] -> [L, 128, nch]
        return np.ascontiguousarray(
            bvec.reshape(L, nch, 128).transpose(0, 2, 1)).astype(f32)

    shared = {
        "wq_in": lhsT_layout(wq_f, 4, 4),
        "wk_in": lhsT_layout(wk_f, 4, 4),
        "wv_in": rhs_layout(wv_f),
        "wo_in": lhsT_layout(wo, 4, 4),
        "w1_in": lhsT_layout(w1_f, 4, 16),
        "w2_in": lhsT_layout(w2, 16, 4),
        "bq_in": b_layout(bq_f, 4),
        "bk_in": b_layout(bk_f, 4),
        "b1_in": b_layout(b1_f, 16),
        "bo_in": b_layout(bo_f, 4),
        "b2_in": b_layout(b2, 4),
    }
    sel = np.zeros((32, 16, 128), dtype=f32)
    for p in range(16):
        k0 = 8 * (p // 4) + 2 * (p % 4)
        sel[k0, p, 0:64] = 1.0
        sel[k0 + 1, p, 64:128] = 1.0
    shared["sel_in"] = sel.astype(BF16)

    xs = x.reshape(NC, BL, N, H)
    eb = np.exp(attn_bias).reshape(NC, BL, HEADS, N, N)
    in_maps = []
    for c in range(NC):
        xT_c = np.ascontiguousarray(
            xs[c].transpose(2, 0, 1).reshape(4, 128, T).transpose(1, 0, 2))
        # [b, h, q, k] -> [b, h, k, q] -> [b, h, ktc, p, q] -> [b*h, p, ktc*q]
        eb_c = eb[c].transpose(0, 1, 3, 2).reshape(BL, HEADS, 2, 128, 256)
        eb_c = np.ascontiguousarray(
            eb_c.transpose(0, 1, 3, 2, 4).reshape(BL * HEADS, 128, 512)).astype(BF16)
        m = {"x_in": xT_c.reshape(128, 4, T), "eb_in": eb_c}
        m.update(shared)
        in_maps.append(m)
    return in_maps


def _run(inputs, trace=False):
    nc = _get_nc()
    in_maps = _prep_inputs(**inputs)
    res = run_bass_kernel_spmd(nc, in_maps, core_ids=list(range(NC)), trace=trace)
    outs = []
    for c in range(NC):
        yT = np.asarray(res.results[c]["y_out"], dtype=np.float32)  # [128, 4, T]
        y = yT.reshape(128, 4, T).transpose(1, 0, 2).reshape(H, BL, N).transpose(1, 2, 0)
        outs.append(y)
    full = np.ascontiguousarray(np.concatenate(outs, axis=0), dtype=np.float32)
    return full, res


def kernel(**inputs):
    full, _ = _run(inputs, trace=False)
    return full


# revision 36
# speedup vs baseline: 1.1753x; 1.1753x over previous
"""Fused 8-layer transformer (pre-LN, MHA + FFN) for TRN2, data-parallel
over batch across 8 NeuronCores.

Layout strategy: feature-major ("transposed") activations resident in SBUF.
x is kept as xT[feature 128-part, hc, token] fp32. All matmuls contract over
features on the partition dim, so no PE transposes are needed anywhere.
LayerNorm stats (over features = partitions) are computed with ones-matmuls
in float32r. Attention works per (batch, head) on 256x256 score tiles in
[k, q] layout; softmax denominators come from an augmented ones-column in v.
exp(attn_bias) is precomputed on host so softmax is exp(s) * eb (no max
subtraction needed: scores are tiny by construction).
"""

import numpy as np
import ml_dtypes
from contextlib import ExitStack

import bass_rust
import concourse.bass as bass
import concourse.tile as tile
from concourse import mybir
from concourse.bass_utils import run_bass_kernel_spmd

BF16 = ml_dtypes.bfloat16

B, N, H, HEADS, DH, F, L = 32, 256, 512, 8, 64, 2048, 8
NC = 8
BL = B // NC            # local batch = 4
T = BL * N              # local tokens = 1024
EPS = 1e-5

FP32 = mybir.dt.float32
BF = mybir.dt.bfloat16
F32R = mybir.dt.float32r


def _legalize_sync(nc):
    # This walrus codegen encodes at most 1 sem wait + 1 sem update per
    # instruction; the Tile scheduler emits more at cross-engine joins.
    # Hoist excess waits onto same-engine NoOps inserted just before
    # (queues are in-order, so this preserves the happens-before) and
    # excess updates onto NoOps just after.
    uid = 0
    for fn in nc.m.functions:
        for blk in fn.blocks:
            out = []
            changed = False
            for ins in blk.instructions:
                si = ins.sync_info
                if si is not None and (len(si.on_wait) > 1 or len(si.on_update) > 1):
                    waits = list(si.on_wait)
                    upds = list(si.on_update)
                    for w in waits[:-1]:
                        uid += 1
                        nop = bass_rust.InstNoOp(name=f"LGLW-{uid}", engine=ins.engine)
                        nop.sync_info = mybir.SyncInfo(on_wait=[w], on_update=[])
                        out.append(nop)
                    post = []
                    if len(upds) > 1:
                        opname = type(ins).__name__
                        assert "DMA" not in opname and "Dma" not in opname, ins.name
                        for u in upds[1:]:
                            uid += 1
                            nop = bass_rust.InstNoOp(
                                name=f"LGLU-{uid}", engine=ins.engine)
                            nop.sync_info = mybir.SyncInfo(on_wait=[], on_update=[u])
                            post.append(nop)
                        upds = upds[:1]
                    ins.sync_info = mybir.SyncInfo(on_wait=waits[-1:], on_update=upds)
                    out.append(ins)
                    out.extend(post)
                    changed = True
                else:
                    out.append(ins)
            if changed:
                blk.instructions = out


def _build_nc():
    nc = bass.Bass("TRN2", target_bir_lowering=False, debug=False)
    AF = mybir.ActivationFunctionType
    OP = mybir.AluOpType

    def din(name, shape, dt):
        return nc.dram_tensor(name, shape, dt, kind="ExternalInput").ap()

    x_in = din("x_in", [128, 4, T], FP32)
    eb_in = din("eb_in", [BL * HEADS, 128, 512], BF)
    wq_in = din("wq_in", [L, 128, 2048], BF)
    wk_in = din("wk_in", [L, 128, 2048], BF)
    wv_in = din("wv_in", [L, 128, 2048], BF)
    wo_in = din("wo_in", [L, 128, 2048], BF)
    w1_in = din("w1_in", [L, 128, 8192], BF)
    w2_in = din("w2_in", [L, 128, 8192], BF)
    bq_in = din("bq_in", [L, 128, 4], FP32)
    bk_in = din("bk_in", [L, 128, 4], FP32)
    b1_in = din("b1_in", [L, 128, 16], FP32)
    bo_in = din("bo_in", [L, 128, 4], FP32)
    b2_in = din("b2_in", [L, 128, 4], FP32)
    sel_in = din("sel_in", [32, 16, 128], BF)
    y_out = nc.dram_tensor("y_out", [128, 4, T], FP32, kind="ExternalOutput").ap()

    with ExitStack() as stk:
        tc = stk.enter_context(tile.TileContext(nc))
        const = stk.enter_context(tc.tile_pool(name="const", bufs=1))
        wts = stk.enter_context(tc.tile_pool(name="wts", bufs=1))
        work = stk.enter_context(tc.tile_pool(name="work", bufs=2))
        smalls = stk.enter_context(tc.tile_pool(name="smalls", bufs=2))
        ebp = stk.enter_context(tc.tile_pool(name="ebp", bufs=3))
        pmm = stk.enter_context(tc.tile_pool(name="pmm", bufs=6, space="PSUM"))
        pacc = stk.enter_context(tc.tile_pool(name="pacc", bufs=2, space="PSUM"))

        xT = const.tile([128, 4, T], FP32, tag="xT")
        ones = const.tile([128, 1], BF, tag="ones")
        nc.vector.memset(ones, 1.0 / H)
        eps_t = const.tile([1, 1], FP32, tag="eps")
        nc.vector.memset(eps_t, EPS)
        ones_r = const.tile([1, 128], BF, tag="ones_r")
        nc.vector.memset(ones_r, 1.0)
        # per-(batch, head-pair) selector for the deferred attention
        # normalization: picks den rows (8b+2hc, 8b+2hc+1) into the two
        # 64-partition halves (host-built; engines can't memset at
        # arbitrary base partitions)
        sel_all = const.tile([32, 16, 128], BF, tag="sel_all")
        nc.sync.dma_start(sel_all, sel_in)
        # v_aug[kt within chunk, token-chunk, head, d | ones-column]
        v_aug = const.tile([128, 8, HEADS, 65], BF, tag="v_aug")
        nc.vector.memset(v_aug[:, :, :, 64:65], 1.0)

        nc.sync.dma_start(xT, x_in)

        def layernorm(tag):
            yT = work.tile([128, 4, T], BF, tag="yT", name=tag, bufs=2)
            for th in range(2):
                tsl = slice(th * 512, (th + 1) * 512)
                ps_s = pmm.tile([128, 512], FP32, tag="mm", name="ps_s")
                ps_q = pmm.tile([128, 512], FP32, tag="mm", name="ps_q")
                for hc in range(4):
                    xb = work.tile([128, 512], BF, tag="xb", name="xb")
                    nc.vector.tensor_scalar_add(xb, xT[:, hc, tsl], 0.0)
                    sq = work.tile([128, 512], BF, tag="sq", name="sq")
                    nc.vector.tensor_mul(sq, xT[:, hc, tsl], xT[:, hc, tsl])
                    nc.tensor.matmul(ps_s[0:1, :], ones, xb,
                                     start=hc == 0, stop=hc == 3)
                    nc.tensor.matmul(ps_q[0:1, :], ones, sq,
                                     start=hc == 0, stop=hc == 3)
                # ps_s[0] = mean, ps_q[0] = E[x^2] (ones pre-scaled by 1/H)
                msq = smalls.tile([1, 512], FP32, tag="msq", name="msq")
                nc.scalar.activation(msq, ps_s[0:1, :], AF.Square)
                var = smalls.tile([1, 512], FP32, tag="var", name="var")
                nc.vector.tensor_sub(var, ps_q[0:1, :], msq)
                # rstd = exp(-0.5*ln(var+eps)); ln/exp share an ACT table set
                # with attention's Exp, and this keeps DVE reciprocal off the
                # critical path
                lnv = smalls.tile([1, 512], FP32, tag="lnv", name="lnv")
                nc.scalar.activation(lnv, var, AF.Ln, bias=eps_t)
                rstd = smalls.tile([1, 512], BF, tag="rstd", name="rstd")
                nc.scalar.activation(rstd, lnv, AF.Exp, scale=-0.5)
                mur = smalls.tile([1, 512], BF, tag="mur", name="mur")
                with nc.allow_low_precision(reason="bf16 bc-matmul operands"):
                    nc.vector.tensor_mul(mur, ps_s[0:1, :], rstd)
                r_bc = pmm.tile([128, 512], FP32, tag="mm", name="r_bc")
                m_bc = pmm.tile([128, 512], FP32, tag="mm", name="m_bc")
                nc.tensor.matmul(r_bc, ones_r, rstd, start=True, stop=True)
                nc.tensor.matmul(m_bc, ones_r, mur, start=True, stop=True)
                for hc in range(4):
                    tmp = work.tile([128, 512], FP32, tag="lntmp", name="lntmp")
                    nc.vector.tensor_mul(tmp, xT[:, hc, tsl], r_bc)
                    nc.vector.tensor_sub(yT[:, hc, tsl], tmp, m_bc)
            return yT

        for l in range(L):
            wq_t = wts.tile([128, 2048], BF, tag="wq", name="wq_t")
            wk_t = wts.tile([128, 2048], BF, tag="wk", name="wk_t")
            wv_t = wts.tile([128, 2048], BF, tag="wv", name="wv_t")
            wo_t = wts.tile([128, 2048], BF, tag="wo", name="wo_t")
            w1_t = wts.tile([128, 8192], BF, tag="w1", name="w1_t", bufs=2)
            w2_t = wts.tile([128, 8192], BF, tag="w2", name="w2_t", bufs=2)
            nc.sync.dma_start(wq_t, wq_in[l])
            nc.sync.dma_start(wk_t, wk_in[l])
            nc.sync.dma_start(wv_t, wv_in[l])
            nc.sync.dma_start(wo_t, wo_in[l])
            nc.sync.dma_start(w1_t, w1_in[l])
            nc.sync.dma_start(w2_t, w2_in[l])
            bq_t = smalls.tile([128, 4], FP32, tag="bq", name="bq_t")
            bk_t = smalls.tile([128, 4], FP32, tag="bk", name="bk_t")
            b1_t = smalls.tile([128, 16], FP32, tag="b1", name="b1_t")
            bo_t = smalls.tile([128, 4], FP32, tag="bo", name="bo_t")
            b2_t = smalls.tile([128, 4], FP32, tag="b2", name="b2_t")
            nc.sync.dma_start(bq_t, bq_in[l])
            nc.sync.dma_start(bk_t, bk_in[l])
            nc.sync.dma_start(b1_t, b1_in[l])
            nc.sync.dma_start(bo_t, bo_in[l])
            nc.sync.dma_start(b2_t, b2_in[l])

            # ---- LN1 ----
            y1 = layernorm("y1T")

            # ---- QKV (q/k feature-major, v token-major augmented) ----
            qT = work.tile([128, 4, T], BF, tag="qT", name="qT", bufs=1)
            kT = work.tile([128, 4, T], BF, tag="kT", name="kT", bufs=1)
            for oc in range(4):
                for th in range(2):
                    tsl = slice(th * 512, (th + 1) * 512)
                    pq = pmm.tile([128, 512], FP32, tag="mm", name="pq")
                    pk = pmm.tile([128, 512], FP32, tag="mm", name="pk")
                    for hc in range(4):
                        wsl = slice((hc * 4 + oc) * 128, (hc * 4 + oc + 1) * 128)
                        nc.tensor.matmul(pq, wq_t[:, wsl], y1[:, hc, tsl],
                                         start=hc == 0, stop=hc == 3)
                        nc.tensor.matmul(pk, wk_t[:, wsl], y1[:, hc, tsl],
                                         start=hc == 0, stop=hc == 3)
                    nc.scalar.activation(qT[:, oc, tsl], pq, AF.Identity,
                                         bias=bq_t[:, oc:oc + 1])
                    nc.scalar.activation(kT[:, oc, tsl], pk, AF.Identity,
                                         bias=bk_t[:, oc:oc + 1])
            for tcc in range(8):
                pv = pmm.tile([128, 512], FP32, tag="mm", name="pv")
                for hc in range(4):
                    nc.tensor.matmul(pv, y1[:, hc, tcc * 128:(tcc + 1) * 128],
                                     wv_t[:, hc * 512:(hc + 1) * 512],
                                     start=hc == 0, stop=hc == 3)
                nc.scalar.activation(v_aug[:, tcc, :, 0:64],
                                     pv.rearrange("p (h d) -> p h d", h=HEADS),
                                     AF.Copy)

            # ---- attention per (batch, head), scores in [k, q] layout ----
            ctxT = work.tile([128, 4, T], BF, tag="ctxT", name="ctxT", bufs=1)
            den = work.tile([32, 256], FP32, tag="den", name="den", bufs=1)
            def attn_post(st):
                # post-ctx ops, emitted one iteration late so they don't sit
                # between chain links on the in-order ACT/DVE queues
                pc, hp, hcq, qsl, bh = st
                nc.vector.tensor_scalar_add(ctxT[hp:hp + 64, hcq, qsl],
                                            pc[0:64, 0:256], 0.0)
                # engines can't write arbitrary base partitions: stage the
                # PSUM den row in SBUF, then DMA-scatter to partition bh
                dstg = smalls.tile([1, 256], FP32, tag="dstg",
                                   name="dstg", bufs=4)
                nc.scalar.activation(dstg, pc[64:65, 0:256], AF.Copy)
                nc.sync.dma_start(den[bh:bh + 1, :], dstg)

            def emit_scores(b, h):
                eb_t = ebp.tile([128, 512], BF, tag="eb", name="eb_t", bufs=4)
                nc.sync.dma_start(eb_t, eb_in[b * HEADS + h])
                hp = (h % 2) * 64
                hcq = h // 2
                qsl = slice(b * 256, (b + 1) * 256)
                ps = pmm.tile([128, 512], FP32, tag="mm", name="ps")
                for ktc in range(2):
                    nc.tensor.matmul(
                        ps[:, ktc * 256:(ktc + 1) * 256],
                        kT[hp:hp + 64, hcq, b * 256 + ktc * 128: b * 256 + (ktc + 1) * 128],
                        qT[hp:hp + 64, hcq, qsl],
                        start=True, stop=True)
                return ps, eb_t, hp, hcq, qsl, b * HEADS + h

            # software-pipelined: scores for iteration i+1 are emitted before
            # ctx of iteration i so the in-order PE queue never head-of-line
            # blocks on the Exp->mul chain
            pairs = [(b, h) for b in range(BL) for h in range(HEADS)]
            cur = emit_scores(*pairs[0])
            prev = None
            for idx, (b, h) in enumerate(pairs):
                ps, eb_t, hp, hcq, qsl, bh = cur
                probs = work.tile([128, 512], BF, tag="probs", name="probs",
                                  bufs=3)
                nc.scalar.activation(probs, ps, AF.Exp)
                nc.vector.tensor_mul(probs, probs, eb_t)
                if idx + 1 < len(pairs):
                    cur = emit_scores(*pairs[idx + 1])
                pc = pmm.tile([128, 512], FP32, tag="mm", name="pc")
                for ktc in range(2):
                    nc.tensor.matmul(pc[0:65, 0:256],
                                     v_aug[:, b * 2 + ktc, h, :],
                                     probs[:, ktc * 256:(ktc + 1) * 256],
                                     start=ktc == 0, stop=ktc == 1)
                if prev is not None:
                    attn_post(prev)
                prev = (pc, hp, hcq, qsl, bh)
            attn_post(prev)
            rden = work.tile([32, 256], BF, tag="rden", name="rden", bufs=1)
            with nc.allow_low_precision(reason="bf16 bc-matmul operands"):
                nc.vector.reciprocal(rden, den)
            for b in range(BL):
                qsl = slice(b * 256, (b + 1) * 256)
                for hcq in range(4):
                    nbc = pmm.tile([128, 512], FP32, tag="mm", name="nbc")
                    nc.tensor.matmul(nbc[:, 0:256], sel_all[:, b * 4 + hcq, :],
                                     rden, start=True, stop=True)
                    nc.vector.tensor_mul(ctxT[:, hcq, qsl], ctxT[:, hcq, qsl],
                                         nbc[:, 0:256])

            # ---- out-proj + residual ----
            for oc in range(4):
                for th in range(2):
                    tsl = slice(th * 512, (th + 1) * 512)
                    po = pmm.tile([128, 512], FP32, tag="mm", name="po")
                    for hc in range(4):
                        wsl = slice((hc * 4 + oc) * 128, (hc * 4 + oc + 1) * 128)
                        nc.tensor.matmul(po, wo_t[:, wsl], ctxT[:, hc, tsl],
                                         start=hc == 0, stop=hc == 3)
                    nc.vector.scalar_tensor_tensor(
                        xT[:, oc, tsl], po, bo_t[:, oc:oc + 1], xT[:, oc, tsl],
                        OP.add, OP.add)

            # ---- LN2 + FFN ----
            y2 = layernorm("y2T")
            for th in range(2):
                tsl = slice(th * 512, (th + 1) * 512)
                # store gelu outputs so FFN2 runs oc-major with a single
                # rotating PSUM accumulator (frees banks for attention)
                g_all = work.tile([128, 16, 512], BF, tag="g_all",
                                  name="g_all", bufs=1)
                for fc in range(16):
                    pf = pmm.tile([128, 512], FP32, tag="mm", name="pf")
                    for hc in range(4):
                        wsl = slice((hc * 16 + fc) * 128, (hc * 16 + fc + 1) * 128)
                        nc.tensor.matmul(pf, w1_t[:, wsl], y2[:, hc, tsl],
                                         start=hc == 0, stop=hc == 3)
                    nc.scalar.activation(g_all[:, fc, :], pf, AF.Gelu,
                                         bias=b1_t[:, fc:fc + 1])
                for oc in range(4):
                    acc = pacc.tile([128, 512], FP32, tag="acc", name="acc",
                                    bufs=2)
                    for fc in range(16):
                        wsl = slice((fc * 4 + oc) * 128, (fc * 4 + oc + 1) * 128)
                        nc.tensor.matmul(acc, w2_t[:, wsl], g_all[:, fc, :],
                                         start=fc == 0, stop=fc == 15)
                    nc.vector.scalar_tensor_tensor(
                        xT[:, oc, tsl], acc, b2_t[:, oc:oc + 1], xT[:, oc, tsl],
                        OP.add, OP.add)

        nc.sync.dma_start(y_out, xT)
    _legalize_sync(nc)
    return nc


_NC_CACHE = {}


def _get_nc():
    if "nc" not in _NC_CACHE:
        _NC_CACHE["nc"] = _build_nc()
    return _NC_CACHE["nc"]


def _prep_inputs(x, attn_bias, ln1_s, ln1_b, wq, bq, wk, bk, wv, bv, wo, bo,
                 ln2_s, ln2_b, w1, b1, w2, b2):
    f32 = np.float32
    asf = lambda a: np.asarray(a, dtype=f32)
    x, attn_bias = asf(x), asf(attn_bias)
    ln1_s, ln1_b, ln2_s, ln2_b = asf(ln1_s), asf(ln1_b), asf(ln2_s), asf(ln2_b)
    wq, wk, wv, wo, w1, w2 = asf(wq), asf(wk), asf(wv), asf(wo), asf(w1), asf(w2)
    bq, bk, bv, bo, b1, b2 = asf(bq), asf(bk), asf(bv), asf(bo), asf(b1), asf(b2)

    scale = f32(DH ** -0.5)
    # fold LN affine into the following matmuls; fold q-scale into wq/bq;
    # fold bv through wo into bo.
    wq_f = ln1_s[:, :, None] * wq * scale
    bq_f = (bq + np.einsum("lh,lho->lo", ln1_b, wq)) * scale
    wk_f = ln1_s[:, :, None] * wk
    bk_f = bk + np.einsum("lh,lho->lo", ln1_b, wk)
    wv_f = ln1_s[:, :, None] * wv
    bv_f = bv + np.einsum("lh,lho->lo", ln1_b, wv)
    bo_f = bo + np.einsum("lh,lho->lo", bv_f, wo)
    w1_f = ln2_s[:, :, None] * w1
    b1_f = b1 + np.einsum("lh,lhf->lf", ln2_b, w1)

    def lhsT_layout(w, ncon, nout):  # w [L, ncon*128, nout*128]
        a = w.reshape(L, ncon, 128, nout, 128).transpose(0, 2, 1, 3, 4)
        return np.ascontiguousarray(a.reshape(L, 128, ncon * nout * 128)).astype(BF16)

    def rhs_layout(w):  # w [L, 512, 512] used as moving operand
        a = w.reshape(L, 4, 128, 512).transpose(0, 2, 1, 3)
        return np.ascontiguousarray(a.reshape(L, 128, 2048)).astype(BF16)

    def b_layout(bvec, nch):  # [L, nch*128] -> [L, 128, nch]
        return np.ascontiguousarray(
            bvec.reshape(L, nch, 128).transpose(0, 2, 1)).astype(f32)

    shared = {
        "wq_in": lhsT_layout(wq_f, 4, 4),
        "wk_in": lhsT_layout(wk_f, 4, 4),
        "wv_in": rhs_layout(wv_f),
        "wo_in": lhsT_layout(wo, 4, 4),
        "w1_in": lhsT_layout(w1_f, 4, 16),
        "w2_in": lhsT_layout(w2, 16, 4),
        "bq_in": b_layout(bq_f, 4),
        "bk_in": b_layout(bk_f, 4),
        "b1_in": b_layout(b1_f, 16),
        "bo_in": b_layout(bo_f, 4),
        "b2_in": b_layout(b2, 4),
    }
    sel = np.zeros((32, 16, 128), dtype=f32)
    for p in range(16):
        k0 = 8 * (p // 4) + 2 * (p % 4)
        sel[k0, p, 0:64] = 1.0
        sel[k0 + 1, p, 64:128] = 1.0
    shared["sel_in"] = sel.astype(BF16)

    xs = x.reshape(NC, BL, N, H)
    eb = np.exp(attn_bias).reshape(NC, BL, HEADS, N, N)
    in_maps = []
    for c in range(NC):
        xT_c = np.ascontiguousarray(
            xs[c].transpose(2, 0, 1).reshape(4, 128, T).transpose(1, 0, 2))
        # [b, h, q, k] -> [b, h, k, q] -> [b, h, ktc, p, q] -> [b*h, p, ktc*q]
        eb_c = eb[c].transpose(0, 1, 3, 2).reshape(BL, HEADS, 2, 128, 256)
        eb_c = np.ascontiguousarray(
            eb_c.transpose(0, 1, 3, 2, 4).reshape(BL * HEADS, 128, 512)).astype(BF16)
        m = {"x_in": xT_c.reshape(128, 4, T), "eb_in": eb_c}
        m.update(shared)
        in_maps.append(m)
    return in_maps


def _run(inputs, trace=False):
    nc = _get_nc()
    in_maps = _prep_inputs(**inputs)
    res = run_bass_kernel_spmd(nc, in_maps, core_ids=list(range(NC)), trace=trace)
    outs = []
    for c in range(NC):
        yT = np.asarray(res.results[c]["y_out"], dtype=np.float32)  # [128, 4, T]
        y = yT.reshape(128, 4, T).transpose(1, 0, 2).reshape(H, BL, N).transpose(1, 2, 0)
        outs.append(y)
    full = np.ascontiguousarray(np.concatenate(outs, axis=0), dtype=np.float32)
    return full, res


def kernel(**inputs):
    full, _ = _run(inputs, trace=False)
    return full
